# revision 35
# baseline (speedup 1.0000x reference)
"""Trainium2 Bass kernel for a 5-layer gated graph conv (GatedGraphConv-style).

Math per layer (reference):
    m    = h @ W[l]                                   # [N, D]
    msgs = m[src] * edge_attr[:, None]                # [E, D]
    agg  = segment_sum(msgs, dst, N)                  # [N, D]
    h    = GRUCell(agg, h)                            # [N, D]

Distribution over 8 NeuronCores (dst-sharded nodes, AllGather of m):
    core c owns NPC=2560 nodes. Per layer it computes m for its slice,
    AllGathers bf16 m, dma_gathers the source rows of its edges, and
    scatter-adds them via PE matmuls against one-hot selection matrices
    S (S[e, dst_local] = edge_attr[e]) that are built ON DEVICE once
    from compact (dst_local, edge_attr) tables. The GRU runs on the
    local slice in transposed layout.

This target is per-instruction-overhead-bound (~25-60us per instruction,
all engines serialized), so the kernel minimizes total instruction count:
single-buffered pools, 8192-index gathers, [128,512] batched copies, and
one strided DMA per phase. m rows are stored p-major (node u at row
(u%128)*NT + u//128 within the core block) so phase A's 10 PSUM copies
land in one contiguous SBUF tile and one linear DMA; the gather indices
compensate on the host.
"""

import math
import os

import numpy as np
import ml_dtypes

import concourse.bass as bass
import concourse.tile as tile
from concourse import bacc, mybir
from concourse.bass_utils import run_bass_kernel_spmd

P = 128
CORES = 8
D = 256
KD = D // P          # 2 contraction chunks
NBW = 512            # node-block width for GRU (PSUM bank = 512 fp32)
G = 64               # edge chunks per gather group (8192 indices/gather)

AF = mybir.ActivationFunctionType
ALU = mybir.AluOpType

bf16 = ml_dtypes.bfloat16


class _Cfg:
    def __init__(self, n_nodes, n_layers, k_ch):
        self.N = n_nodes
        self.L = n_layers
        self.NPC = int(math.ceil(n_nodes / CORES / NBW)) * NBW  # padded nodes/core
        self.N_PAD = self.NPC * CORES
        self.NT = self.NPC // P    # dst tiles per core
        self.NB = self.NPC // NBW  # node blocks per core
        self.K_CH = k_ch           # chunks per dst tile (uniform across cores)
        self.NCH = self.NT * k_ch
        self.NG = (self.NCH + G - 1) // G
        self.NCH_PAD = self.NG * G


def _preprocess(cfg, edge_index, edge_attr):
    """Vectorized edge sharding: per-core (dloc, ea) chunk tables + gather idx."""
    src = np.asarray(edge_index[0], dtype=np.int64)
    dst = np.asarray(edge_index[1], dtype=np.int64)
    ea = np.asarray(edge_attr, dtype=np.float32)

    core = (dst // cfg.NPC).astype(np.int32)
    dl = (dst % cfg.NPC).astype(np.int32)

    # Global sort by (core, dst_local) groups edges per core in dst order.
    order = np.argsort(core * np.int32(cfg.NPC) + dl, kind="stable")
    src_s, dl_s, ea_s, core_s = src[order].astype(np.int32), dl[order], ea[order], core[order]

    # chunk tables, uniform K_CH across (core, tile)
    tile_g = core_s * cfg.NT + dl_s // P  # global tile id 0..CORES*NT
    counts = np.bincount(tile_g, minlength=CORES * cfg.NT)
    k_ch = max(1, int(np.max((counts + P - 1) // P)))
    cfg.K_CH = k_ch
    cfg.NCH = cfg.NT * k_ch
    cfg.NG = (cfg.NCH + G - 1) // G
    cfg.NCH_PAD = cfg.NG * G

    starts = np.zeros(CORES * cfg.NT + 1, np.int64)
    np.cumsum(counts, out=starts[1:])
    rank = np.arange(len(src_s)) - starts[tile_g]  # rank of edge within its tile

    # position of each edge in the padded per-core [NCH_PAD*P] edge array
    tile_in_core = tile_g % cfg.NT
    pos = core_s * (cfg.NCH_PAD * P) + tile_in_core * (k_ch * P) + rank

    src_pad = np.zeros(CORES * cfg.NCH_PAD * P, np.int32)
    ea_pad = np.zeros(CORES * cfg.NCH_PAD * P, np.float32)
    dl_pad = np.zeros(CORES * cfg.NCH_PAD * P, np.float32)
    src_pad[pos] = src_s
    ea_pad[pos] = ea_s
    dl_pad[pos] = dl_s % P  # dst within tile, 0..127

    # m rows are stored p-major per core: node u -> row c*NPC + (u%P)*NT + u//P
    s_c = src_pad // cfg.NPC
    s_l = src_pad % cfg.NPC
    grow = s_c * cfg.NPC + (s_l % P) * cfg.NT + s_l // P

    outs = []
    for c in range(CORES):
        sl = slice(c * cfg.NCH_PAD * P, (c + 1) * cfg.NCH_PAD * P)
        # dloc/ea as [P, NCH_PAD]: column q holds chunk q's 128 edges
        dloc = np.ascontiguousarray(dl_pad[sl].reshape(cfg.NCH_PAD, P).T)
        eav = np.ascontiguousarray(ea_pad[sl].reshape(cfg.NCH_PAD, P).T)
        # gather idx packing: per group of G*P indices: [G*8, 16].T, tiled to 128
        gsrc = grow[sl].astype(np.int16)
        pk = gsrc.reshape(cfg.NG, G * 8, 16).transpose(0, 2, 1).reshape(cfg.NG * 16, G * 8)
        idx = np.zeros((P, cfg.NG * G * 8), np.int16)
        for g in range(cfg.NG):
            blk = pk[g * 16 : (g + 1) * 16]
            idx[:, g * G * 8 : (g + 1) * G * 8] = np.tile(blk, (8, 1))
        outs.append({"dloc": dloc, "ea": eav, "idx": idx})
    return outs


def _build(cfg):
    nc = bacc.Bacc(
        "TRN2",
        target_bir_lowering=False,
        debug=False,
        num_devices=CORES,
        num_swdge_queues=4,
    )
    dt = mybir.dt
    NPC, NT, NB, L = cfg.NPC, cfg.NT, cfg.NB, cfg.L
    K_CH, NCH, NG, NCH_PAD = cfg.K_CH, cfg.NCH, cfg.NG, cfg.NCH_PAD
    bf16_io = os.environ.get("GGC2_BF16", "1") == "1"
    io_dt = dt.bfloat16 if bf16_io else dt.float32
    reps = int(os.environ.get("GGC2_REPS", "1"))

    # replicated weights (W | gru | ident) are column-sharded across cores
    # and reassembled on device with one AllGather: 1/8th the upload bytes.
    CWT = L * KD * D + KD * 2 * 3 * D + P  # 5760 merged const columns
    assert CWT % CORES == 0
    CW = CWT // CORES

    xT_in = nc.dram_tensor("xT", [KD, P, NPC], io_dt, kind="ExternalInput").ap()
    cw_in = nc.dram_tensor("cw", [P, CW], dt.bfloat16, kind="ExternalInput").ap()
    b_in = nc.dram_tensor("bias", [P, 8], dt.float32, kind="ExternalInput").ap()
    dloc_in = nc.dram_tensor("dloc", [P, NCH_PAD], dt.float32, kind="ExternalInput").ap()
    ea_in = nc.dram_tensor("ea", [P, NCH_PAD], dt.float32, kind="ExternalInput").ap()
    idx_in = nc.dram_tensor("idx", [16, NG * G * 8], dt.int16, kind="ExternalInput").ap()
    out_hT = nc.dram_tensor("out_hT", [KD, P, NPC], io_dt, kind="ExternalOutput").ap()

    cw_c = nc.dram_tensor("cw_c", [P, CW], dt.bfloat16)
    cw_full = nc.dram_tensor("cw_full", [CORES, P, CW], dt.bfloat16, addr_space="Shared")

    S_dram = nc.dram_tensor("S_dram", [NG, P, G, P], dt.bfloat16)
    # single shared collective buffers; reuse serializes collectives via deps.
    # m_c rows are p-major: byte-identical to the msb SBUF tile.
    m_c = nc.dram_tensor("m_c", [P, NT * D], dt.bfloat16)
    m_full = nc.dram_tensor("m_full", [cfg.N_PAD, D], dt.bfloat16, addr_space="Shared")
    rg = [list(range(CORES))]

    with tile.TileContext(nc) as tc:
        with (
            tc.tile_pool(name="const", bufs=1) as constp,
            tc.tile_pool(name="work", bufs=1) as wp,
            tc.tile_pool(name="sS", bufs=1) as sp,
            tc.tile_pool(name="msg", bufs=1) as msgp,
            tc.tile_pool(name="psA", bufs=1, space="PSUM") as psA,
            tc.tile_pool(name="psS", bufs=1, space="PSUM") as psS,
            tc.tile_pool(name="psT", bufs=1, space="PSUM") as psT,
            tc.tile_pool(name="psG", bufs=1, space="PSUM") as psG,
        ):
            # ---- constants ----
            cwm = constp.tile([P, CWT], dt.bfloat16, tag="cwm", name="cwm")
            nc.sync.dma_start(cwm[:, :CW], cw_in[:])  # bounce own slice via SBUF
            nc.sync.dma_start(cw_c[:], cwm[:, :CW])
            nc.gpsimd.collective_compute(
                "AllGather", ALU.bypass, replica_groups=rg,
                ins=[cw_c[:]], outs=[cw_full[:]],
            )
            for r in range(CORES):
                nc.sync.dma_start(cwm[:, r * CW : (r + 1) * CW], cw_full[r])
            GRU_BASE = L * KD * D
            IDENT_BASE = CWT - P
            b_sb = constp.tile([P, 8], dt.float32, tag="bias", name="b_sb")
            nc.sync.dma_start(b_sb[:], b_in[:])
            dloc_sb = constp.tile([P, NCH_PAD], dt.float32, tag="dloc", name="dloc_sb")
            nc.sync.dma_start(dloc_sb[:], dloc_in[:])
            ea_sb = constp.tile([P, NCH_PAD], dt.float32, tag="ea", name="ea_sb")
            nc.sync.dma_start(ea_sb[:], ea_in[:])
            idx_sb = constp.tile([P, NG * G * 8], dt.int16, tag="idx", name="idx_sb")
            nc.sync.dma_start(idx_sb[0:16, :], idx_in[:])
            for r in range(1, 8):
                nc.sync.dma_start(idx_sb[16 * r : 16 * r + 16, :], idx_in[:])

            def wof(l, k):  # W[l] k-chunk columns in cwm
                return (l * KD + k) * D

            def gof(k, which, gch):  # gru weight columns in cwm: which 0=wih 1=whh
                return GRU_BASE + k * (2 * 3 * D) + which * (3 * D) + gch * P

            # ---- h init (transposed layout: [128, KD*NPC] fp32 + bf16) ----
            h_t = wp.tile([P, KD * NPC], dt.float32, tag="h_t", name="h_t")
            h_b = wp.tile([P, KD * NPC], dt.bfloat16, tag="h_b", name="h_b")
            if bf16_io:
                for k in range(KD):
                    nc.sync.dma_start(h_b[:, k * NPC : (k + 1) * NPC], xT_in[k])
                nc.scalar.activation(h_t[:], h_b[:], AF.Copy)
            else:
                for k in range(KD):
                    nc.sync.dma_start(h_t[:, k * NPC : (k + 1) * NPC], xT_in[k])
                nc.scalar.activation(h_b[:], h_t[:], AF.Copy)

            # ---- build S on device (once): 2 broadcast DVE ops per group ----
            from concourse.bass import broadcast_tensor_aps

            iota_g = constp.tile([P, G, P], dt.bfloat16, tag="iotag", name="iota_g")
            nc.gpsimd.iota(iota_g[:], pattern=[[0, G], [1, P]], base=0,
                           channel_multiplier=0, allow_small_or_imprecise_dtypes=True)
            dloc_b = constp.tile([P, NCH_PAD], dt.bfloat16, tag="dlocb", name="dloc_b")
            nc.scalar.activation(dloc_b[:], dloc_sb[:], AF.Copy)
            ea_b = constp.tile([P, NCH_PAD], dt.bfloat16, tag="eab", name="ea_b")
            nc.scalar.activation(ea_b[:], ea_sb[:], AF.Copy)
            S_bld = sp.tile([P, G, P], dt.bfloat16, tag="S", name="S_bld")
            for g in range(NG):
                d3 = dloc_b[:, g * G : (g + 1) * G, None]
                i3, d3b = broadcast_tensor_aps(iota_g[:], d3)
                nc.vector.tensor_tensor(S_bld[:], i3, d3b, op=ALU.is_equal)
                e3 = ea_b[:, g * G : (g + 1) * G, None]
                s3, e3b = broadcast_tensor_aps(S_bld[:], e3)
                nc.vector.tensor_tensor(S_bld[:], s3, e3b, op=ALU.mult)
                nc.sync.dma_start(S_dram[g], S_bld[:])

            msb = wp.tile([P, NT * D], dt.bfloat16, tag="msb", name="msb")
            nat = wp.tile([P, NT * D], dt.bfloat16, tag="nat", name="nat")
            aggT = wp.tile([P, KD * NPC], dt.bfloat16, tag="aggT", name="aggT")
            rz = [wp.tile([P, NBW], dt.float32, tag=f"rz{i}", name=f"rz{i}") for i in range(4)]
            ntl = [wp.tile([P, NBW], dt.float32, tag=f"nt{i}", name=f"nt{i}") for i in range(2)]
            tmp = wp.tile([P, NBW], dt.float32, tag="tmp", name="tmp")

            for l in [ll % L for ll in range(L * reps)]:
                # ---- A: m = h @ W[l] -> p-major rows in m_c ----
                for pair in range(NT // 2):
                    ps = psA.tile([P, 2 * D], dt.float32, tag="psA", name="psA")
                    for half in range(2):
                        t = 2 * pair + half
                        for k in range(KD):
                            nc.tensor.matmul(
                                ps[:, half * D : (half + 1) * D],
                                lhsT=h_b[:, k * NPC + t * P : k * NPC + (t + 1) * P],
                                rhs=cwm[:, wof(l, k) : wof(l, k) + D],
                                start=(k == 0),
                                stop=(k == KD - 1),
                            )
                    nc.scalar.activation(
                        msb[:, pair * 2 * D : (pair + 1) * 2 * D], ps[:], AF.Copy
                    )
                # one linear DMA: msb[p, t*D:(t+1)*D] == m row (p*NT + t)
                nc.sync.dma_start(m_c[:], msb[:])

                # ---- B: AllGather m ----
                if os.environ.get("GGC2_SIM", "0") == "1":
                    # timing stand-in: keep the msb -> m_full dep, no collective
                    nc.sync.dma_start(m_full[0:P, :], msb[:, 0:D])
                else:
                    nc.gpsimd.collective_compute(
                        "AllGather", ALU.bypass, replica_groups=rg,
                        ins=[m_c[:]], outs=[m_full[:]],
                    )

                # ---- C: gather + scatter matmuls -> nat (p-major agg), then transpose ----
                for g in range(NG):
                    mt = msgp.tile([P, G, D], dt.bfloat16, tag="mt", name="mt")
                    nc.gpsimd.dma_gather(
                        out_ap=mt[:],
                        in_ap=m_full[:],
                        idxs_ap=idx_sb[:, g * G * 8 : (g + 1) * G * 8],
                        num_idxs=G * P,
                        num_idxs_reg=G * P,
                        elem_size=D,
                        single_packet=False,
                        queue_num=g % 4,
                    )
                    st = sp.tile([P, G, P], dt.bfloat16, tag="S", name="st")
                    nc.sync.dma_start(st[:], S_dram[g])
                    for j in range(G):
                        q = g * G + j
                        if q >= NCH:
                            break
                        t, jj = divmod(q, K_CH)
                        half = t % 2
                        if jj == 0 and half == 0:
                            ps_sc = psS.tile([P, 2 * D], dt.float32, tag="psS", name="psS")
                        nc.tensor.matmul(
                            ps_sc[:, half * D : (half + 1) * D],
                            lhsT=st[:, j, :],
                            rhs=mt[:, j, :],
                            start=(jj == 0),
                            stop=(jj == K_CH - 1),
                        )
                        if jj == K_CH - 1 and half == 1:
                            nc.scalar.activation(
                                nat[:, (t - 1) * D : (t + 1) * D], ps_sc[:], AF.Copy
                            )
                # transposes: nat[p, t*D + k*P ...] -> aggT[d, k*NPC + t*P ...]
                for pair in range(NT // 2):
                    ps_t = psT.tile([P, 4 * P], dt.bfloat16, tag="psT", name="psT")
                    for half in range(2):
                        t = 2 * pair + half
                        for k in range(KD):
                            nc.tensor.transpose(
                                ps_t[:, (2 * k + half) * P : (2 * k + half + 1) * P],
                                nat[:, t * D + k * P : t * D + (k + 1) * P],
                                cwm[:, IDENT_BASE : IDENT_BASE + P],
                            )
                    for k in range(KD):
                        nc.scalar.activation(
                            aggT[:, k * NPC + pair * 2 * P : k * NPC + (pair + 1) * 2 * P],
                            ps_t[:, 2 * k * P : (2 * k + 2) * P],
                            AF.Copy,
                        )

                # ---- D: GRU on local slice (transposed layout) ----
                for nb in range(NB):

                    def mm_gates(ps, gch, rhs_tile, which, start, stop):
                        for k in range(KD):
                            o = k * NPC + nb * NBW
                            nc.tensor.matmul(
                                ps[:],
                                lhsT=cwm[:, gof(k, which, gch) : gof(k, which, gch) + P],
                                rhs=rhs_tile[:, o : o + NBW],
                                start=(start and k == 0),
                                stop=(stop and k == KD - 1),
                            )

                    for gch in range(4):  # r0 r1 z0 z1
                        ps = psG.tile([P, NBW], dt.float32, tag="psG", name="psG")
                        mm_gates(ps, gch, aggT, 0, True, False)
                        mm_gates(ps, gch, h_b, 1, False, True)
                        nc.scalar.activation(
                            rz[gch][:], ps[:], AF.Sigmoid, bias=b_sb[:, gch : gch + 1]
                        )
                    for k2 in range(2):  # n gate halves
                        ps_i = psG.tile([P, NBW], dt.float32, tag="psG", name="ps_i")
                        mm_gates(ps_i, 4 + k2, aggT, 0, True, True)
                        ps_h = psG.tile([P, NBW], dt.float32, tag="psG2", name="ps_h")
                        mm_gates(ps_h, 4 + k2, h_b, 1, True, True)
                        nc.vector.tensor_scalar_add(
                            tmp[:], ps_h[:], b_sb[:, 6 + k2 : 7 + k2]
                        )
                        nc.vector.tensor_tensor(tmp[:], rz[k2][:], tmp[:], op=ALU.mult)
                        nc.vector.tensor_tensor(tmp[:], ps_i[:], tmp[:], op=ALU.add)
                        nc.scalar.activation(
                            ntl[k2][:], tmp[:], AF.Tanh, bias=b_sb[:, 4 + k2 : 5 + k2]
                        )
                    for k in range(KD):  # h' = n + z*(h-n)
                        o = k * NPC + nb * NBW
                        hsl = h_t[:, o : o + NBW]
                        nc.vector.tensor_tensor(tmp[:], hsl, ntl[k][:], op=ALU.subtract)
                        nc.vector.tensor_tensor(tmp[:], rz[2 + k][:], tmp[:], op=ALU.mult)
                        nc.vector.tensor_tensor(hsl, ntl[k][:], tmp[:], op=ALU.add)
                        nc.scalar.activation(h_b[:, o : o + NBW], hsl, AF.Copy)

            for k in range(KD):
                src = h_b if bf16_io else h_t
                nc.sync.dma_start(out_hT[k], src[:, k * NPC : (k + 1) * NPC])

    nc.compile()
    return nc


_BUILD_CACHE = {}


def _get_built(key, cfg):
    if key not in _BUILD_CACHE:
        _BUILD_CACHE[key] = _build(cfg)
    return _BUILD_CACHE[key]


class _FastResults:
    """Shim matching BassKernelResults' fields used by test harnesses."""

    def __init__(self, results):
        self.results = results
        self.exec_time_ns = None
        self.mean_exec_time_ns = None
        self.instructions_and_trace = None
        self.profile_json = None


_JIT_CACHE = {}


def _run_pjrt_fast(nc, in_maps, n_cores):
    """run_bass_via_pjrt equivalent, but the donated output buffers are
    created on-device (jnp.zeros with a sharding) instead of being uploaded
    as host zeros — saves shipping the full output size through the tunnel."""
    import jax
    import jax.numpy as jnp
    from jax.sharding import Mesh, NamedSharding, PartitionSpec
    from jax.experimental.shard_map import shard_map
    from concourse import bass2jax

    bass2jax.install_neuronx_cc_hook()
    key = id(nc)
    if key not in _JIT_CACHE:
        in_names, out_names, out_avals = [], [], []
        for alloc in nc.m.functions[0].allocations:
            if not isinstance(alloc, mybir.MemoryLocationSet):
                continue
            name = alloc.memorylocations[0].name
            if alloc.kind == "ExternalInput":
                if nc.partition_id_tensor is None or name != nc.partition_id_tensor.name:
                    in_names.append(name)
            elif alloc.kind == "ExternalOutput":
                shape = tuple(alloc.tensor_shape)
                dtype = mybir.dt.np(alloc.dtype)
                out_names.append(name)
                out_avals.append(jax.core.ShapedArray(shape, dtype))
        n_params = len(in_names)
        all_in_names = list(in_names) + list(out_names)
        if nc.partition_id_tensor is not None:
            all_in_names.append(nc.partition_id_tensor.name)

        def _body(*args):
            operands = list(args)
            if nc.partition_id_tensor is not None:
                operands.append(bass2jax.partition_id_tensor())
            outs = bass2jax._bass_exec_p.bind(
                *operands,
                out_avals=tuple(out_avals),
                in_names=tuple(all_in_names),
                out_names=tuple(out_names),
                lowering_input_output_aliases=(),
                sim_require_finite=True,
                sim_require_nnan=True,
                nc=nc,
            )
            return tuple(outs)

        devices = jax.devices()[:n_cores]
        mesh = Mesh(np.asarray(devices), ("core",))
        n_outs = len(out_avals)
        donate = tuple(range(n_params, n_params + n_outs))
        in_specs = (PartitionSpec("core"),) * (n_params + n_outs)
        out_specs = (PartitionSpec("core"),) * n_outs
        fn = jax.jit(
            shard_map(
                _body, mesh=mesh, in_specs=in_specs, out_specs=out_specs,
                check_rep=False,
            ),
            donate_argnums=donate,
            keep_unused=True,
        )
        _JIT_CACHE[key] = (fn, in_names, out_names, out_avals, mesh)
    fn, in_names, out_names, out_avals, mesh = _JIT_CACHE[key]
    concat_in = [
        np.concatenate([np.asarray(m[name]) for m in in_maps], axis=0)
        for name in in_names
    ]
    sh = NamedSharding(mesh, PartitionSpec("core"))
    zeros_dev = [
        jnp.zeros((n_cores * av.shape[0], *av.shape[1:]), av.dtype, device=sh)
        for av in out_avals
    ]
    out_arrs = fn(*concat_in, *zeros_dev)
    return _FastResults(
        [
            {
                name: np.asarray(out_arrs[i]).reshape(n_cores, *out_avals[i].shape)[c]
                for i, name in enumerate(out_names)
            }
            for c in range(n_cores)
        ]
    )


def build_in_maps(cfg, pre, x, weight, w_ih, w_hh, b_ih, b_hh):
    n_layers = weight.shape[0]
    x_pad = np.zeros((cfg.N_PAD, D), np.float32)
    x_pad[: cfg.N] = np.asarray(x, np.float32)
    # W_sb columns: (l,k) chunk at (l*KD+k)*D
    W_host = np.ascontiguousarray(
        np.asarray(weight, np.float32)
        .reshape(n_layers, KD, P, D)
        .transpose(2, 0, 1, 3)
        .reshape(P, n_layers * KD * D)
        .astype(bf16)
    )
    wihT = np.asarray(w_ih, np.float32).T.reshape(KD, P, 3 * D)
    whhT = np.asarray(w_hh, np.float32).T.reshape(KD, P, 3 * D)
    gru_host = np.ascontiguousarray(
        np.concatenate([wihT, whhT], axis=2).transpose(1, 0, 2).reshape(P, KD * 2 * 3 * D).astype(bf16)
    )
    b_ih = np.asarray(b_ih, np.float32)
    b_hh = np.asarray(b_hh, np.float32)
    bias = np.zeros((P, 8), np.float32)
    bias[:, 0:4] = (b_ih + b_hh)[: 2 * D].reshape(4, P).T
    bias[:, 4:6] = b_ih[2 * D :].reshape(2, P).T
    bias[:, 6:8] = b_hh[2 * D :].reshape(2, P).T
    ident = np.eye(P, dtype=bf16)
    cw_all = np.concatenate([W_host, gru_host, ident], axis=1)  # [P, CWT]
    CW = cw_all.shape[1] // CORES

    bf16_io = os.environ.get("GGC2_BF16", "1") == "1"
    if bf16_io:
        x_pad = x_pad.astype(bf16)
    in_maps = []
    for c in range(CORES):
        xT_c = np.ascontiguousarray(
            x_pad[c * cfg.NPC : (c + 1) * cfg.NPC].T.reshape(KD, P, cfg.NPC)
        )
        in_maps.append(
            {
                "xT": xT_c,
                "cw": np.ascontiguousarray(cw_all[:, c * CW : (c + 1) * CW]),
                "bias": bias,
                "dloc": pre[c]["dloc"],
                "ea": pre[c]["ea"],
                "idx": np.ascontiguousarray(pre[c]["idx"][:16]),
            }
        )
    return in_maps


def run(x, edge_index, edge_attr, weight, w_ih, w_hh, b_ih, b_hh, trace=False):
    n_nodes = x.shape[0]
    n_layers = weight.shape[0]
    assert x.shape[1] == D and w_ih.shape == (3 * D, D)

    cfg = _Cfg(n_nodes, n_layers, 1)
    pre = _preprocess(cfg, edge_index, edge_attr)
    in_maps = build_in_maps(cfg, pre, x, weight, w_ih, w_hh, b_ih, b_hh)

    key = (n_nodes, n_layers, cfg.K_CH, cfg.NG,
           os.environ.get("GGC2_BF16", "1"), os.environ.get("GGC2_REPS", "1"),
           os.environ.get("GGC2_SIM", "0"))
    nc = _get_built(key, cfg)

    res = None
    if os.environ.get("GGC2_FASTDISPATCH", "1") == "1":
        try:
            res = _run_pjrt_fast(nc, in_maps, CORES)
        except Exception:
            res = None
    if res is None:
        try:
            res = run_bass_kernel_spmd(nc, in_maps, list(range(CORES)), trace=trace)
        except ModuleNotFoundError:
            res = run_bass_kernel_spmd(nc, in_maps, list(range(CORES)), trace=False)

    h = np.zeros((cfg.N_PAD, D), np.float32)
    for c in range(CORES):
        o = res.results[c]["out_hT"]  # [KD, P, NPC]
        h[c * cfg.NPC : (c + 1) * cfg.NPC] = o.reshape(D, cfg.NPC).T.astype(np.float32)
    return h[:n_nodes], res


def kernel(**inputs):
    h, _ = run(**inputs)
    return h


# revision 39
# speedup vs baseline: 1.0888x; 1.0888x over previous
"""Trainium2 Bass kernel for a 5-layer gated graph conv (GatedGraphConv-style).

Math per layer (reference):
    m    = h @ W[l]                                   # [N, D]
    msgs = m[src] * edge_attr[:, None]                # [E, D]
    agg  = segment_sum(msgs, dst, N)                  # [N, D]
    h    = GRUCell(agg, h)                            # [N, D]

Distribution over 8 NeuronCores (dst-sharded nodes, AllGather of m):
    core c owns NPC=2560 nodes. Per layer it computes m for its slice,
    AllGathers bf16 m, dma_gathers the source rows of its edges, and
    scatter-adds them via PE matmuls against one-hot selection matrices
    S (S[e, dst_local] = edge_attr[e]) that are built ON DEVICE once
    from compact (dst_local, edge_attr) tables. The GRU runs on the
    local slice in transposed layout.

This target is per-instruction-overhead-bound (~25-60us per instruction,
all engines serialized), so the kernel minimizes total instruction count:
single-buffered pools, 8192-index gathers, [128,512] batched copies, and
one strided DMA per phase. m rows are stored p-major (node u at row
(u%128)*NT + u//128 within the core block) so phase A's 10 PSUM copies
land in one contiguous SBUF tile and one linear DMA; the gather indices
compensate on the host.
"""

import math
import os

import numpy as np
import ml_dtypes

import concourse.bass as bass
import concourse.tile as tile
from concourse import bacc, mybir
from concourse.bass_utils import run_bass_kernel_spmd

P = 128
CORES = 8
D = 256
KD = D // P          # 2 contraction chunks
NBW = 512            # node-block width for GRU (PSUM bank = 512 fp32)
G = 64               # edge chunks per gather group (8192 indices/gather)

AF = mybir.ActivationFunctionType
ALU = mybir.AluOpType

bf16 = ml_dtypes.bfloat16


class _Cfg:
    def __init__(self, n_nodes, n_layers, k_ch):
        self.N = n_nodes
        self.L = n_layers
        self.NPC = int(math.ceil(n_nodes / CORES / NBW)) * NBW  # padded nodes/core
        self.N_PAD = self.NPC * CORES
        self.NT = self.NPC // P    # dst tiles per core
        self.NB = self.NPC // NBW  # node blocks per core
        self.K_CH = k_ch           # chunks per dst tile (uniform across cores)
        self.NCH = self.NT * k_ch
        self.NG = (self.NCH + G - 1) // G
        self.NCH_PAD = self.NG * G


def _preprocess(cfg, edge_index, edge_attr):
    """Vectorized edge sharding: per-core (dloc, ea) chunk tables + gather idx."""
    src = np.asarray(edge_index[0], dtype=np.int64)
    dst = np.asarray(edge_index[1], dtype=np.int64)
    ea = np.asarray(edge_attr, dtype=np.float32)

    core = (dst // cfg.NPC).astype(np.int32)
    dl = (dst % cfg.NPC).astype(np.int32)

    # Global sort by (core, dst_local) groups edges per core in dst order.
    order = np.argsort(core * np.int32(cfg.NPC) + dl, kind="stable")
    src_s, dl_s, ea_s, core_s = src[order].astype(np.int32), dl[order], ea[order], core[order]

    # chunk tables, uniform K_CH across (core, tile)
    tile_g = core_s * cfg.NT + dl_s // P  # global tile id 0..CORES*NT
    counts = np.bincount(tile_g, minlength=CORES * cfg.NT)
    k_ch = max(1, int(np.max((counts + P - 1) // P)))
    cfg.K_CH = k_ch
    cfg.NCH = cfg.NT * k_ch
    cfg.NG = (cfg.NCH + G - 1) // G
    cfg.NCH_PAD = cfg.NG * G

    starts = np.zeros(CORES * cfg.NT + 1, np.int64)
    np.cumsum(counts, out=starts[1:])
    rank = np.arange(len(src_s)) - starts[tile_g]  # rank of edge within its tile

    # position of each edge in the padded per-core [NCH_PAD*P] edge array
    tile_in_core = tile_g % cfg.NT
    pos = core_s * (cfg.NCH_PAD * P) + tile_in_core * (k_ch * P) + rank

    src_pad = np.zeros(CORES * cfg.NCH_PAD * P, np.int32)
    ea_pad = np.zeros(CORES * cfg.NCH_PAD * P, np.float32)
    dl_pad = np.zeros(CORES * cfg.NCH_PAD * P, np.float32)
    src_pad[pos] = src_s
    ea_pad[pos] = ea_s
    dl_pad[pos] = dl_s % P  # dst within tile, 0..127

    # m rows are stored p-major per core: node u -> row c*NPC + (u%P)*NT + u//P
    s_c = src_pad // cfg.NPC
    s_l = src_pad % cfg.NPC
    grow = s_c * cfg.NPC + (s_l % P) * cfg.NT + s_l // P

    outs = []
    for c in range(CORES):
        sl = slice(c * cfg.NCH_PAD * P, (c + 1) * cfg.NCH_PAD * P)
        # dloc/ea as [P, NCH_PAD]: column q holds chunk q's 128 edges
        dloc = np.ascontiguousarray(dl_pad[sl].reshape(cfg.NCH_PAD, P).T.astype(bf16))
        eav = np.ascontiguousarray(ea_pad[sl].reshape(cfg.NCH_PAD, P).T.astype(bf16))
        # gather idx packing: per group of G*P indices: [G*8, 16].T, tiled to 128
        gsrc = grow[sl].astype(np.int16)
        pk = gsrc.reshape(cfg.NG, G * 8, 16).transpose(0, 2, 1).reshape(cfg.NG * 16, G * 8)
        idx = np.zeros((P, cfg.NG * G * 8), np.int16)
        for g in range(cfg.NG):
            blk = pk[g * 16 : (g + 1) * 16]
            idx[:, g * G * 8 : (g + 1) * G * 8] = np.tile(blk, (8, 1))
        outs.append({"dloc": dloc, "ea": eav, "idx": idx})
    return outs


def _build(cfg):
    nc = bacc.Bacc(
        "TRN2",
        target_bir_lowering=False,
        debug=False,
        num_devices=CORES,
        num_swdge_queues=4,
    )
    dt = mybir.dt
    NPC, NT, NB, L = cfg.NPC, cfg.NT, cfg.NB, cfg.L
    K_CH, NCH, NG, NCH_PAD = cfg.K_CH, cfg.NCH, cfg.NG, cfg.NCH_PAD
    bf16_io = os.environ.get("GGC2_BF16", "1") == "1"
    io_dt = dt.bfloat16 if bf16_io else dt.float32
    reps = int(os.environ.get("GGC2_REPS", "1"))

    # replicated weights (W | gru | ident) are column-sharded across cores
    # and reassembled on device with one AllGather: 1/8th the upload bytes.
    CWT = L * KD * D + KD * 2 * 3 * D + P  # 5760 merged const columns
    assert CWT % CORES == 0
    CW = CWT // CORES

    xT_in = nc.dram_tensor("xT", [KD, P, NPC], io_dt, kind="ExternalInput").ap()
    cw_in = nc.dram_tensor("cw", [P, CW], dt.bfloat16, kind="ExternalInput").ap()
    b_in = nc.dram_tensor("bias", [P, 8], dt.float32, kind="ExternalInput").ap()
    dloc_in = nc.dram_tensor("dloc", [P, NCH_PAD], dt.bfloat16, kind="ExternalInput").ap()
    ea_in = nc.dram_tensor("ea", [P, NCH_PAD], dt.bfloat16, kind="ExternalInput").ap()
    idx_in = nc.dram_tensor("idx", [16, NG * G * 8], dt.int16, kind="ExternalInput").ap()
    out_hT = nc.dram_tensor("out_hT", [KD, P, NPC], io_dt, kind="ExternalOutput").ap()

    cw_c = nc.dram_tensor("cw_c", [P, CW], dt.bfloat16)
    cw_full = nc.dram_tensor("cw_full", [CORES, P, CW], dt.bfloat16, addr_space="Shared")

    S_dram = nc.dram_tensor("S_dram", [NG, P, G, P], dt.bfloat16)
    # single shared collective buffers; reuse serializes collectives via deps.
    # m_c rows are p-major: byte-identical to the msb SBUF tile.
    m_c = nc.dram_tensor("m_c", [P, NT * D], dt.bfloat16)
    m_full = nc.dram_tensor("m_full", [cfg.N_PAD, D], dt.bfloat16, addr_space="Shared")
    rg = [list(range(CORES))]

    with tile.TileContext(nc) as tc:
        with (
            tc.tile_pool(name="const", bufs=1) as constp,
            tc.tile_pool(name="work", bufs=1) as wp,
            tc.tile_pool(name="sS", bufs=1) as sp,
            tc.tile_pool(name="msg", bufs=1) as msgp,
            tc.tile_pool(name="psA", bufs=1, space="PSUM") as psA,
            tc.tile_pool(name="psS", bufs=1, space="PSUM") as psS,
            tc.tile_pool(name="psT", bufs=1, space="PSUM") as psT,
            tc.tile_pool(name="psG", bufs=1, space="PSUM") as psG,
        ):
            # ---- constants ----
            cwm = constp.tile([P, CWT], dt.bfloat16, tag="cwm", name="cwm")
            nc.sync.dma_start(cwm[:, :CW], cw_in[:])  # bounce own slice via SBUF
            nc.sync.dma_start(cw_c[:], cwm[:, :CW])
            nc.gpsimd.collective_compute(
                "AllGather", ALU.bypass, replica_groups=rg,
                ins=[cw_c[:]], outs=[cw_full[:]],
            )
            for r in range(CORES):
                nc.sync.dma_start(cwm[:, r * CW : (r + 1) * CW], cw_full[r])
            GRU_BASE = L * KD * D
            IDENT_BASE = CWT - P
            b_sb = constp.tile([P, 8], dt.float32, tag="bias", name="b_sb")
            nc.sync.dma_start(b_sb[:], b_in[:])
            dloc_sb = constp.tile([P, NCH_PAD], dt.bfloat16, tag="dloc", name="dloc_sb")
            nc.sync.dma_start(dloc_sb[:], dloc_in[:])
            ea_sb = constp.tile([P, NCH_PAD], dt.bfloat16, tag="ea", name="ea_sb")
            nc.sync.dma_start(ea_sb[:], ea_in[:])
            idx_sb = constp.tile([P, NG * G * 8], dt.int16, tag="idx", name="idx_sb")
            nc.sync.dma_start(idx_sb[0:16, :], idx_in[:])
            for r in range(1, 8):
                nc.sync.dma_start(idx_sb[16 * r : 16 * r + 16, :], idx_in[:])

            def wof(l, k):  # W[l] k-chunk columns in cwm
                return (l * KD + k) * D

            def gof(k, which, gch):  # gru weight columns in cwm: which 0=wih 1=whh
                return GRU_BASE + k * (2 * 3 * D) + which * (3 * D) + gch * P

            # ---- h init (transposed layout: [128, KD*NPC] fp32 + bf16) ----
            h_t = wp.tile([P, KD * NPC], dt.float32, tag="h_t", name="h_t")
            h_b = wp.tile([P, KD * NPC], dt.bfloat16, tag="h_b", name="h_b")
            if bf16_io:
                for k in range(KD):
                    nc.sync.dma_start(h_b[:, k * NPC : (k + 1) * NPC], xT_in[k])
                nc.scalar.activation(h_t[:], h_b[:], AF.Copy)
            else:
                for k in range(KD):
                    nc.sync.dma_start(h_t[:, k * NPC : (k + 1) * NPC], xT_in[k])
                nc.scalar.activation(h_b[:], h_t[:], AF.Copy)

            # ---- build S on device (once): 2 broadcast DVE ops per group ----
            from concourse.bass import broadcast_tensor_aps

            iota_g = constp.tile([P, G, P], dt.bfloat16, tag="iotag", name="iota_g")
            nc.gpsimd.iota(iota_g[:], pattern=[[0, G], [1, P]], base=0,
                           channel_multiplier=0, allow_small_or_imprecise_dtypes=True)
            S_bld = sp.tile([P, G, P], dt.bfloat16, tag="S", name="S_bld")
            for g in range(NG):
                d3 = dloc_sb[:, g * G : (g + 1) * G, None]
                i3, d3b = broadcast_tensor_aps(iota_g[:], d3)
                nc.vector.tensor_tensor(S_bld[:], i3, d3b, op=ALU.is_equal)
                e3 = ea_sb[:, g * G : (g + 1) * G, None]
                s3, e3b = broadcast_tensor_aps(S_bld[:], e3)
                nc.vector.tensor_tensor(S_bld[:], s3, e3b, op=ALU.mult)
                nc.sync.dma_start(S_dram[g], S_bld[:])

            msb = wp.tile([P, NT * D], dt.bfloat16, tag="msb", name="msb")
            nat = wp.tile([P, NT * D], dt.bfloat16, tag="nat", name="nat")
            aggT = wp.tile([P, KD * NPC], dt.bfloat16, tag="aggT", name="aggT")
            rz = [wp.tile([P, NBW], dt.float32, tag=f"rz{i}", name=f"rz{i}") for i in range(4)]
            ntl = [wp.tile([P, NBW], dt.float32, tag=f"nt{i}", name=f"nt{i}") for i in range(2)]
            tmp = wp.tile([P, NBW], dt.float32, tag="tmp", name="tmp")
            ps_half = [None, None]  # scatter PSUM chain per tile-pair half

            for l in [ll % L for ll in range(L * reps)]:
                # ---- A: m = h @ W[l] -> p-major rows in m_c ----
                for pair in range(NT // 2):
                    ps = psA.tile([P, 2 * D], dt.float32, tag="psA", name="psA")
                    for half in range(2):
                        t = 2 * pair + half
                        for k in range(KD):
                            nc.tensor.matmul(
                                ps[:, half * D : (half + 1) * D],
                                lhsT=h_b[:, k * NPC + t * P : k * NPC + (t + 1) * P],
                                rhs=cwm[:, wof(l, k) : wof(l, k) + D],
                                start=(k == 0),
                                stop=(k == KD - 1),
                            )
                    nc.scalar.activation(
                        msb[:, pair * 2 * D : (pair + 1) * 2 * D], ps[:], AF.Copy
                    )
                # one linear DMA: msb[p, t*D:(t+1)*D] == m row (p*NT + t)
                nc.sync.dma_start(m_c[:], msb[:])

                # ---- B: AllGather m ----
                if os.environ.get("GGC2_SIM", "0") == "1":
                    # timing stand-in: keep the msb -> m_full dep, no collective
                    nc.sync.dma_start(m_full[0:P, :], msb[:, 0:D])
                else:
                    nc.gpsimd.collective_compute(
                        "AllGather", ALU.bypass, replica_groups=rg,
                        ins=[m_c[:]], outs=[m_full[:]],
                    )

                # ---- C: gather + scatter matmuls -> nat (p-major agg), then transpose ----
                for g in range(NG):
                    mt = msgp.tile([P, G, D], dt.bfloat16, tag="mt", name="mt")
                    nc.gpsimd.dma_gather(
                        out_ap=mt[:],
                        in_ap=m_full[:],
                        idxs_ap=idx_sb[:, g * G * 8 : (g + 1) * G * 8],
                        num_idxs=G * P,
                        num_idxs_reg=G * P,
                        elem_size=D,
                        single_packet=False,
                        queue_num=g % 4,
                    )
                    st = sp.tile([P, G, P], dt.bfloat16, tag="S", name="st")
                    nc.sync.dma_start(st[:], S_dram[g])
                    # Interleave each tile pair's two accumulation chains across
                    # two PSUM banks: back-to-back matmuls into the SAME bank
                    # cost ~60us vs ~44us when alternating (accum hazard).
                    qlo, qhi = g * G, min((g + 1) * G, NCH)
                    qs = []
                    for pr in range(qlo // (2 * K_CH), (qhi - 1) // (2 * K_CH) + 1):
                        c0 = [q for q in range((2 * pr) * K_CH, (2 * pr + 1) * K_CH)
                              if qlo <= q < qhi]
                        c1 = [q for q in range((2 * pr + 1) * K_CH, (2 * pr + 2) * K_CH)
                              if qlo <= q < qhi]
                        for i in range(max(len(c0), len(c1))):
                            if i < len(c0):
                                qs.append(c0[i])
                            if i < len(c1):
                                qs.append(c1[i])
                    for q in qs:
                        j = q - g * G
                        t, jj = divmod(q, K_CH)
                        half = t % 2
                        if jj == 0:
                            ps_half[half] = psS.tile(
                                [P, 2 * D], dt.float32, tag=f"psS{half}", name=f"psS{half}"
                            )
                        nc.tensor.matmul(
                            ps_half[half][:, :D],
                            lhsT=st[:, j, :],
                            rhs=mt[:, j, :],
                            start=(jj == 0),
                            stop=(jj == K_CH - 1),
                        )
                        if jj == K_CH - 1:
                            nc.scalar.activation(
                                nat[:, t * D : (t + 1) * D], ps_half[half][:, :D], AF.Copy
                            )
                # transposes: nat[p, t*D + k*P ...] -> aggT[d, k*NPC + t*P ...]
                for pair in range(NT // 2):
                    ps_t = psT.tile([P, 4 * P], dt.bfloat16, tag="psT", name="psT")
                    for half in range(2):
                        t = 2 * pair + half
                        for k in range(KD):
                            nc.tensor.transpose(
                                ps_t[:, (2 * k + half) * P : (2 * k + half + 1) * P],
                                nat[:, t * D + k * P : t * D + (k + 1) * P],
                                cwm[:, IDENT_BASE : IDENT_BASE + P],
                            )
                    for k in range(KD):
                        nc.scalar.activation(
                            aggT[:, k * NPC + pair * 2 * P : k * NPC + (pair + 1) * 2 * P],
                            ps_t[:, 2 * k * P : (2 * k + 2) * P],
                            AF.Copy,
                        )

                # ---- D: GRU on local slice (transposed layout) ----
                for nb in range(NB):

                    def mm_gates(ps, gch, rhs_tile, which, start, stop):
                        for k in range(KD):
                            o = k * NPC + nb * NBW
                            nc.tensor.matmul(
                                ps[:],
                                lhsT=cwm[:, gof(k, which, gch) : gof(k, which, gch) + P],
                                rhs=rhs_tile[:, o : o + NBW],
                                start=(start and k == 0),
                                stop=(stop and k == KD - 1),
                            )

                    def mm_one(ps, gch, which, rhs_tile, k, start, stop):
                        o = k * NPC + nb * NBW
                        nc.tensor.matmul(
                            ps[:],
                            lhsT=cwm[:, gof(k, which, gch) : gof(k, which, gch) + P],
                            rhs=rhs_tile[:, o : o + NBW],
                            start=start,
                            stop=stop,
                        )

                    for gp in range(2):  # gate pairs (r0,r1), (z0,z1) bank-interleaved
                        pa = psG.tile([P, NBW], dt.float32, tag="psG", name="pa")
                        pb = psG.tile([P, NBW], dt.float32, tag="psG2", name="pb")
                        for which, rhs_t in ((0, aggT), (1, h_b)):
                            for k in range(KD):
                                first = which == 0 and k == 0
                                last = which == 1 and k == KD - 1
                                mm_one(pa, 2 * gp, which, rhs_t, k, first, last)
                                mm_one(pb, 2 * gp + 1, which, rhs_t, k, first, last)
                        for half, ps in ((0, pa), (1, pb)):
                            gch = 2 * gp + half
                            nc.scalar.activation(
                                rz[gch][:], ps[:], AF.Sigmoid, bias=b_sb[:, gch : gch + 1]
                            )
                    for k2 in range(2):  # n gate halves, gi/gh bank-interleaved
                        ps_i = psG.tile([P, NBW], dt.float32, tag="psG", name="ps_i")
                        ps_h = psG.tile([P, NBW], dt.float32, tag="psG2", name="ps_h")
                        for k in range(KD):
                            mm_one(ps_i, 4 + k2, 0, aggT, k, k == 0, k == KD - 1)
                            mm_one(ps_h, 4 + k2, 1, h_b, k, k == 0, k == KD - 1)
                        nc.vector.tensor_scalar_add(
                            tmp[:], ps_h[:], b_sb[:, 6 + k2 : 7 + k2]
                        )
                        nc.vector.tensor_tensor(tmp[:], rz[k2][:], tmp[:], op=ALU.mult)
                        nc.vector.tensor_tensor(tmp[:], ps_i[:], tmp[:], op=ALU.add)
                        nc.scalar.activation(
                            ntl[k2][:], tmp[:], AF.Tanh, bias=b_sb[:, 4 + k2 : 5 + k2]
                        )
                    for k in range(KD):  # h' = n + z*(h-n)
                        o = k * NPC + nb * NBW
                        hsl = h_t[:, o : o + NBW]
                        nc.vector.tensor_tensor(tmp[:], hsl, ntl[k][:], op=ALU.subtract)
                        nc.vector.tensor_tensor(tmp[:], rz[2 + k][:], tmp[:], op=ALU.mult)
                        nc.vector.tensor_tensor(hsl, ntl[k][:], tmp[:], op=ALU.add)
                        nc.scalar.activation(h_b[:, o : o + NBW], hsl, AF.Copy)

            for k in range(KD):
                src = h_b if bf16_io else h_t
                nc.sync.dma_start(out_hT[k], src[:, k * NPC : (k + 1) * NPC])

    nc.compile()
    return nc


_BUILD_CACHE = {}


def _get_built(key, cfg):
    if key not in _BUILD_CACHE:
        _BUILD_CACHE[key] = _build(cfg)
    return _BUILD_CACHE[key]


class _FastResults:
    """Shim matching BassKernelResults' fields used by test harnesses."""

    def __init__(self, results):
        self.results = results
        self.exec_time_ns = None
        self.mean_exec_time_ns = None
        self.instructions_and_trace = None
        self.profile_json = None


_JIT_CACHE = {}


def _run_pjrt_fast(nc, in_maps, n_cores):
    """run_bass_via_pjrt equivalent, but the donated output buffers are
    created on-device (jnp.zeros with a sharding) instead of being uploaded
    as host zeros — saves shipping the full output size through the tunnel."""
    import jax
    import jax.numpy as jnp
    from jax.sharding import Mesh, NamedSharding, PartitionSpec
    from jax.experimental.shard_map import shard_map
    from concourse import bass2jax

    bass2jax.install_neuronx_cc_hook()
    key = id(nc)
    if key not in _JIT_CACHE:
        in_names, out_names, out_avals = [], [], []
        for alloc in nc.m.functions[0].allocations:
            if not isinstance(alloc, mybir.MemoryLocationSet):
                continue
            name = alloc.memorylocations[0].name
            if alloc.kind == "ExternalInput":
                if nc.partition_id_tensor is None or name != nc.partition_id_tensor.name:
                    in_names.append(name)
            elif alloc.kind == "ExternalOutput":
                shape = tuple(alloc.tensor_shape)
                dtype = mybir.dt.np(alloc.dtype)
                out_names.append(name)
                out_avals.append(jax.core.ShapedArray(shape, dtype))
        n_params = len(in_names)
        all_in_names = list(in_names) + list(out_names)
        if nc.partition_id_tensor is not None:
            all_in_names.append(nc.partition_id_tensor.name)

        def _body(*args):
            operands = list(args)
            if nc.partition_id_tensor is not None:
                operands.append(bass2jax.partition_id_tensor())
            outs = bass2jax._bass_exec_p.bind(
                *operands,
                out_avals=tuple(out_avals),
                in_names=tuple(all_in_names),
                out_names=tuple(out_names),
                lowering_input_output_aliases=(),
                sim_require_finite=True,
                sim_require_nnan=True,
                nc=nc,
            )
            return tuple(outs)

        devices = jax.devices()[:n_cores]
        mesh = Mesh(np.asarray(devices), ("core",))
        n_outs = len(out_avals)
        donate = tuple(range(n_params, n_params + n_outs))
        in_specs = (PartitionSpec("core"),) * (n_params + n_outs)
        out_specs = (PartitionSpec("core"),) * n_outs
        fn = jax.jit(
            shard_map(
                _body, mesh=mesh, in_specs=in_specs, out_specs=out_specs,
                check_rep=False,
            ),
            donate_argnums=donate,
            keep_unused=True,
        )
        _JIT_CACHE[key] = (fn, in_names, out_names, out_avals, mesh)
    fn, in_names, out_names, out_avals, mesh = _JIT_CACHE[key]
    concat_in = [
        np.concatenate([np.asarray(m[name]) for m in in_maps], axis=0)
        for name in in_names
    ]
    sh = NamedSharding(mesh, PartitionSpec("core"))
    zeros_dev = [
        jnp.zeros((n_cores * av.shape[0], *av.shape[1:]), av.dtype, device=sh)
        for av in out_avals
    ]
    out_arrs = fn(*concat_in, *zeros_dev)
    return _FastResults(
        [
            {
                name: np.asarray(out_arrs[i]).reshape(n_cores, *out_avals[i].shape)[c]
                for i, name in enumerate(out_names)
            }
            for c in range(n_cores)
        ]
    )


def build_in_maps(cfg, pre, x, weight, w_ih, w_hh, b_ih, b_hh):
    n_layers = weight.shape[0]
    x_pad = np.zeros((cfg.N_PAD, D), np.float32)
    x_pad[: cfg.N] = np.asarray(x, np.float32)
    # W_sb columns: (l,k) chunk at (l*KD+k)*D
    W_host = np.ascontiguousarray(
        np.asarray(weight, np.float32)
        .reshape(n_layers, KD, P, D)
        .transpose(2, 0, 1, 3)
        .reshape(P, n_layers * KD * D)
        .astype(bf16)
    )
    wihT = np.asarray(w_ih, np.float32).T.reshape(KD, P, 3 * D)
    whhT = np.asarray(w_hh, np.float32).T.reshape(KD, P, 3 * D)
    gru_host = np.ascontiguousarray(
        np.concatenate([wihT, whhT], axis=2).transpose(1, 0, 2).reshape(P, KD * 2 * 3 * D).astype(bf16)
    )
    b_ih = np.asarray(b_ih, np.float32)
    b_hh = np.asarray(b_hh, np.float32)
    bias = np.zeros((P, 8), np.float32)
    bias[:, 0:4] = (b_ih + b_hh)[: 2 * D].reshape(4, P).T
    bias[:, 4:6] = b_ih[2 * D :].reshape(2, P).T
    bias[:, 6:8] = b_hh[2 * D :].reshape(2, P).T
    ident = np.eye(P, dtype=bf16)
    cw_all = np.concatenate([W_host, gru_host, ident], axis=1)  # [P, CWT]
    CW = cw_all.shape[1] // CORES

    bf16_io = os.environ.get("GGC2_BF16", "1") == "1"
    if bf16_io:
        x_pad = x_pad.astype(bf16)
    in_maps = []
    for c in range(CORES):
        xT_c = np.ascontiguousarray(
            x_pad[c * cfg.NPC : (c + 1) * cfg.NPC].T.reshape(KD, P, cfg.NPC)
        )
        in_maps.append(
            {
                "xT": xT_c,
                "cw": np.ascontiguousarray(cw_all[:, c * CW : (c + 1) * CW]),
                "bias": bias,
                "dloc": pre[c]["dloc"],
                "ea": pre[c]["ea"],
                "idx": np.ascontiguousarray(pre[c]["idx"][:16]),
            }
        )
    return in_maps


def run(x, edge_index, edge_attr, weight, w_ih, w_hh, b_ih, b_hh, trace=False):
    n_nodes = x.shape[0]
    n_layers = weight.shape[0]
    assert x.shape[1] == D and w_ih.shape == (3 * D, D)

    cfg = _Cfg(n_nodes, n_layers, 1)
    pre = _preprocess(cfg, edge_index, edge_attr)
    in_maps = build_in_maps(cfg, pre, x, weight, w_ih, w_hh, b_ih, b_hh)

    key = (n_nodes, n_layers, cfg.K_CH, cfg.NG,
           os.environ.get("GGC2_BF16", "1"), os.environ.get("GGC2_REPS", "1"),
           os.environ.get("GGC2_SIM", "0"))
    nc = _get_built(key, cfg)

    res = None
    if os.environ.get("GGC2_FASTDISPATCH", "1") == "1":
        try:
            res = _run_pjrt_fast(nc, in_maps, CORES)
        except Exception:
            res = None
    if res is None:
        try:
            res = run_bass_kernel_spmd(nc, in_maps, list(range(CORES)), trace=trace)
        except ModuleNotFoundError:
            res = run_bass_kernel_spmd(nc, in_maps, list(range(CORES)), trace=False)

    h = np.zeros((cfg.N_PAD, D), np.float32)
    for c in range(CORES):
        o = res.results[c]["out_hT"]  # [KD, P, NPC]
        h[c * cfg.NPC : (c + 1) * cfg.NPC] = o.reshape(D, cfg.NPC).T.astype(np.float32)
    return h[:n_nodes], res


def kernel(**inputs):
    h, _ = run(**inputs)
    return h
